# revision 7
# baseline (speedup 1.0000x reference)
"""BigResNet Trainium2 kernel.

Computation (see reference): x:[65536,100]; 100 blocks of
(10x Linear(100,100)+ReLU) with a residual add per block; final Linear(100,10).

Strategy:
- Data-parallel over the batch: 8 cores x 8192 rows each.
- Activations live in SBUF transposed: [D=100 (+1 ones row), batch]. The
  contraction dim D sits on SBUF partitions for both matmul operands, so no
  transposes are needed anywhere in the layer chain.
- Bias is folded into the matmul via a constant ones-row at partition 100 and
  an extra weight row (K=101).
- Weights are host-side rearranged to [101, block, layer*100] so each block's
  weights DMA as 101 partitions x 4000B contiguous lines.
- Matmul dtype float32r (fp32 truncated to FP22 inside the PE): full PE rate,
  ~2^-12 relative precision, fp32 accumulate in PSUM.
- ReLU drains PSUM->SBUF, split between ScalarE (activation) and VectorE
  (tensor_scalar_max). The block-residual is fused into the last layer's
  drain as one VectorE scalar_tensor_tensor: out = max(z,0) + x.
"""

import sys

sys.path.insert(0, "/opt/trn_rl_repo")

import numpy as np
from contextlib import ExitStack

import concourse.bass as bass
import concourse.bacc as bacc
import concourse.tile as tile
from concourse import mybir
from concourse.bass_utils import run_bass_kernel_spmd

N_BLOCKS = 100
LAYERS_PER_BLOCK = 10
D = 100
D_OUT = 10
BATCH = 65536
N_CORES = 8
B_CORE = BATCH // N_CORES  # 8192 batch columns per core
KAUG = D + 1  # 100 weight rows + 1 bias row

F32 = mybir.dt.float32
F32R = mybir.dt.float32r

# Column-group size for the PSUM->SBUF drain ops (ReLU / residual).
GROUP = 1024
N_GROUPS = B_CORE // GROUP  # 8
MM_N = 512  # max moving-operand free dim for fp32
MM_PER_GROUP = GROUP // MM_N  # 2

# Per (layer, group) ReLU engine assignment: ScalarE gets ~42% of groups to
# balance ACT (N+352)/1.2 against DVE (N+~150)/1.4 per-op cost.
def _use_act(layer: int, group: int) -> bool:
    if layer == LAYERS_PER_BLOCK - 1:
        return False  # residual layer: fused op is DVE-only
    return group % 2 == (0 if layer % 2 == 0 else 1) and group < 7


def _build(n_blocks: int = N_BLOCKS, b_core: int = B_CORE):
    n_groups = b_core // GROUP
    nc = bacc.Bacc("TRN2", target_bir_lowering=False, debug=False,
                   num_devices=N_CORES)

    xt = nc.dram_tensor("xt", [KAUG, b_core], F32R, kind="ExternalInput").ap()
    wa = nc.dram_tensor("wa", [KAUG, n_blocks, LAYERS_PER_BLOCK * D], F32R,
                        kind="ExternalInput").ap()
    wf = nc.dram_tensor("wf", [KAUG, D_OUT], F32R, kind="ExternalInput").ap()
    out = nc.dram_tensor("out", [D_OUT, b_core], F32,
                         kind="ExternalOutput").ap()

    with tile.TileContext(nc) as tc, ExitStack() as ctx:
        acts = ctx.enter_context(tc.tile_pool(name="acts", bufs=1))
        wpool = ctx.enter_context(tc.tile_pool(name="w", bufs=2))
        wfpool = ctx.enter_context(tc.tile_pool(name="wf", bufs=1))
        opool = ctx.enter_context(tc.tile_pool(name="o", bufs=1))
        psum = ctx.enter_context(tc.tile_pool(name="ps", bufs=3, space="PSUM"))
        fpsum = ctx.enter_context(tc.tile_pool(name="fps", bufs=2, space="PSUM"))

        # Three resident activation buffers, rotated across blocks.
        bufs = [acts.tile([KAUG, b_core], F32R, tag=f"act{i}", name=f"act{i}")
                for i in range(3)]
        # x lands in bufs[0]; host ships the ones-row as row 100 of xt.
        nc.gpsimd.dma_start(bufs[0][:, :], xt[:, :])
        # The temp buffers need their ones-row too (ReLU only writes rows
        # 0:100); copy it from xt's ones-row.
        nc.gpsimd.dma_start(bufs[1][D:KAUG, :], xt[D:KAUG, :])
        nc.gpsimd.dma_start(bufs[2][D:KAUG, :], xt[D:KAUG, :])

        wf_sb = wfpool.tile([KAUG, D_OUT], F32R)
        nc.gpsimd.dma_start(wf_sb[:, :], wf[:, :])

        x_buf, t1, t2 = bufs[0], bufs[1], bufs[2]
        for bl in range(n_blocks):
            wt = wpool.tile([KAUG, LAYERS_PER_BLOCK * D], F32R, tag="wt")
            nc.gpsimd.dma_start(wt[:, :], wa[:, bl, :])

            cur = x_buf
            for layer in range(LAYERS_PER_BLOCK):
                w_l = wt[:, layer * D:(layer + 1) * D]
                last = layer == LAYERS_PER_BLOCK - 1
                dst = t2 if last else (t1 if layer % 2 == 0 else t2)
                for g in range(n_groups):
                    ps = psum.tile([D, GROUP], F32, tag="ps")
                    for h in range(MM_PER_GROUP):
                        c0 = g * GROUP + h * MM_N
                        nc.tensor.matmul(
                            ps[:, h * MM_N:(h + 1) * MM_N],
                            w_l,
                            cur[:, c0:c0 + MM_N],
                            start=True, stop=True,
                        )
                    gs = slice(g * GROUP, (g + 1) * GROUP)
                    if last:
                        # x_new = max(z, 0) + x, one DVE op from PSUM
                        nc.vector.scalar_tensor_tensor(
                            dst[0:D, gs], ps[:, :], 0.0, x_buf[0:D, gs],
                            op0=mybir.AluOpType.max, op1=mybir.AluOpType.add)
                    elif _use_act(layer, g):
                        nc.scalar.activation(
                            dst[0:D, gs], ps[:, :],
                            mybir.ActivationFunctionType.Relu)
                    else:
                        nc.vector.tensor_scalar_max(dst[0:D, gs], ps[:, :], 0.0)
                cur = dst
            # rotate: new x is t2 (holds x+y); old x becomes scratch
            x_buf, t1, t2 = t2, x_buf, t1

        # Final Linear(100 -> 10): psum [10, 512] tiles, copy to SBUF, DMA out.
        out_sb = opool.tile([D_OUT, b_core], F32)
        n_fin = b_core // MM_N
        for t in range(n_fin):
            ps = fpsum.tile([D_OUT, MM_N], F32, tag="fps")
            c0 = t * MM_N
            nc.tensor.matmul(ps[:, :], wf_sb[:, :],
                             x_buf[:, c0:c0 + MM_N],
                             start=True, stop=True)
            cs = slice(c0, c0 + MM_N)
            if t % 2 == 0:
                nc.vector.tensor_copy(out_sb[:, cs], ps[:, :])
            else:
                nc.scalar.copy(out_sb[:, cs], ps[:, :])
        nc.gpsimd.dma_start(out[:, :], out_sb[:, :])

    nc.compile()
    return nc


def _prep_inputs(x, W, b, Wf, bf):
    """Host-side reshape/augment; returns per-core input maps."""
    # wa[i, bl, l*100+o]: i<100 -> W[bl,l,o,i]; i==100 -> b[bl,l,o]
    wa = np.empty((KAUG, N_BLOCKS, LAYERS_PER_BLOCK * D), np.float32)
    wt = np.ascontiguousarray(W.transpose(3, 0, 1, 2))  # [i, bl, l, o]
    wa[:D] = wt.reshape(D, N_BLOCKS, LAYERS_PER_BLOCK * D)
    wa[D] = b.reshape(N_BLOCKS, LAYERS_PER_BLOCK * D)

    wfa = np.empty((KAUG, D_OUT), np.float32)
    wfa[:D] = Wf.T
    wfa[D] = bf

    xt = np.empty((KAUG, BATCH), np.float32)
    xt[:D] = x.T
    xt[D] = 1.0

    in_maps = []
    for c in range(N_CORES):
        sl = slice(c * B_CORE, (c + 1) * B_CORE)
        in_maps.append({
            "xt": np.ascontiguousarray(xt[:, sl]),
            "wa": wa,
            "wf": wfa,
        })
    return in_maps


_CACHED_NC = None


def kernel(x, W, b, Wf, bf, _trace=False, _trace_kwargs=None):
    global _CACHED_NC
    x = np.asarray(x, np.float32)
    in_maps = _prep_inputs(np.asarray(x, np.float32), np.asarray(W, np.float32),
                           np.asarray(b, np.float32), np.asarray(Wf, np.float32),
                           np.asarray(bf, np.float32))
    if _CACHED_NC is None:
        _CACHED_NC = _build()
    nc = _CACHED_NC
    kw = dict(_trace_kwargs or {})
    res = run_bass_kernel_spmd(nc, in_maps, core_ids=list(range(N_CORES)),
                               trace=_trace, **kw)
    outs = [res.results[c]["out"] for c in range(N_CORES)]  # [10, 8192] each
    full = np.concatenate(outs, axis=1).T  # [65536, 10]
    if _trace:
        kernel.last_results = res
    return np.ascontiguousarray(full)
